# revision 27
# baseline (speedup 1.0000x reference)
"""Trainium2 Bass kernel: 2-layer GAT (PyG GATConv, heads=1) + per-node actor
MLP + candidate softmax, SPMD across 8 NeuronCores.

Strategy (dst-sharded data parallel):
  - Symmetrized edges (self loops handled separately), partitioned by dst
    across 8 cores, grouped into 128-dst blocks, GG blocks per gather group.
    Per (group, segment) the edges are packed block-major into one padded
    run of 128-edge chunks; a chunk may span adjacent blocks, handled by
    per-(chunk, block) matmul instances.
  - Node table per layer: bf16 [z(128) | e_src | 1.0 | pad] rows (512B).
    Per edge, dma_gather pulls the src row (segment-relative int16 idx).
  - Host precomputes the transposed one-hot (dst-on-partitions) per
    instance, streamed bf16; one matmul per instance gathers e_dst onto
    edge partitions. alpha = es + ed batched per group: DVE add + DVE
    leaky-relu + one Act exp (keeps Act inside one act-table set).
  - ohw[e,dst] = (iota==d_local)*ex built in one bf16 DVE op per instance;
    one PE matmul per instance accumulates numerator AND denominator
    (table's ones column) into the block psum. Self-loop contributions are
    added per block via a diagonal matmul from the local z-shard staging.
  - Epilogue per block: h = elu(num/den + b) (+1 trick), PE transpose,
    projection to next layer's table row + the shard-local ed table.
    Phase 0 and the layer boundary AllGather the bf16 node tables.
  - Scores are per-node scalars -> AllGather 400KB -> candidate gather +
    grouped softmax over vm=16, sharded over decisions.
"""

import math
import os
import sys

sys.path.insert(0, "/opt/trn_rl_repo")

import ml_dtypes
import numpy as np

import concourse.bass as bass
import concourse.mybir as mybir
import concourse.tile as tile
from concourse import bacc
from concourse.bass import IndirectOffsetOnAxis
from concourse.bass_utils import run_bass_kernel_spmd

F32 = mybir.dt.float32
I32 = mybir.dt.int32
I16 = mybir.dt.int16
BF16 = mybir.dt.bfloat16
ALU = mybir.AluOpType
ACTF = mybir.ActivationFunctionType
BF = ml_dtypes.bfloat16

NEG_SLOPE = 0.2
P = 128
SEGB = 32             # blocks per table segment (int16-indexable slices)
TW = 256              # bf16 table row: z(128) | es | 1.0 | pad  (512B)
GG = 4                # blocks per gather group (must divide SEGB)


# ----------------------------------------------------------------- host prep
def _schedule(edge_index, N, n_cores):
    """Common chunk/instance schedule + per-core index arrays.

    Node table is split into NSEG per-segment tensors; segment s holds every
    core's blocks [32s, 32s+32) packed core-major, so a segment's AllGather
    can fire as soon as those blocks are computed on all cores.
    """
    NSH = N // n_cores
    NBLK = math.ceil(NSH / P)
    NSEG = math.ceil(NBLK / SEGB)
    e0 = edge_index[0].astype(np.int64)
    e1 = edge_index[1].astype(np.int64)
    src = np.concatenate([e0, e1])
    dst = np.concatenate([e1, e0])

    # per-core rows per segment, padded to whole blocks (seg 3 is partial)
    seg_blks_ = [min(SEGB, NBLK - s * SEGB) for s in range(NSEG)]
    spp = [sb * P for sb in seg_blks_]        # padded per-core rows / seg
    # labels: node n -> (seg, idx); layer-1 table is rank-major per seg,
    # layer-0 table is per-core rotated (own core's rows first).
    allc = src // NSH
    allr = src % NSH
    alls = np.minimum(allr // (SEGB * P), NSEG - 1)
    segoff = allr - alls * SEGB * P           # offset within own seg rows
    sppv = np.array(spp, dtype=np.int64)
    lab1 = allc * sppv[alls] + segoff

    # bucket edges: per core, per block, per segment (src-sorted)
    percore = []
    for c in range(n_cores):
        m = (dst >= c * NSH) & (dst < (c + 1) * NSH)
        s_c, d_c = src[m], dst[m] - c * NSH
        sseg_c = alls[m]
        sidx1_c = lab1[m]
        # layer-0 table: per-core rotated AND partition-major within each
        # 8-block projection stage (contiguous 4KB-per-partition writes)
        jl = segoff[m] // P                  # block within segment
        pp = segoff[m] % P                   # row within block
        jn0 = np.where(sseg_c < NSEG - 1, 8, seg_blks_[-1])
        st0 = jl // 8
        jj0 = jl % 8
        sidx0_c = (((allc[m] - c) % n_cores) * sppv[sseg_c]
                   + st0 * jn0 * P + pp * jn0 + jj0)
        o = np.lexsort((s_c, d_c // P))
        d_c = d_c[o]
        sseg_c = sseg_c[o]
        sidx0_c, sidx1_c = sidx0_c[o], sidx1_c[o]
        blk = d_c // P
        bs = np.searchsorted(blk, np.arange(NBLK), side="left")
        be = np.searchsorted(blk, np.arange(NBLK), side="right")
        per_blk = []
        for b in range(NBLK):
            sl = slice(bs[b], be[b])
            seg = sseg_c[sl]
            segs = []
            for s in range(NSEG):
                sm = seg == s
                segs.append((sidx0_c[sl][sm], sidx1_c[sl][sm], d_c[sl][sm]))
            per_blk.append(segs)
        percore.append(per_blk)

    # common per (group, seg) padded run lengths (shared by all cores) and
    # per (group, seg, block) edge counts per core to derive instance spans
    ngrp = math.ceil(NBLK / GG)
    run_len = np.zeros((ngrp, NSEG), dtype=np.int64)   # padded (x128)
    for gi in range(ngrp):
        b0 = gi * GG
        g = min(GG, NBLK - b0)
        for s in range(NSEG):
            mx = 0
            for c in range(n_cores):
                tot = sum(len(percore[c][b0 + bb][s][0]) for bb in range(g))
                mx = max(mx, tot)
            run_len[gi, s] = math.ceil(mx / P) * P if mx else 0

    # groups meta: per group, per seg: chunk col base; chunk count
    groups = []          # (b0, g, segs=[(s, cb_chunk, nch)], gc)
    kk = 0
    for gi in range(ngrp):
        b0 = gi * GG
        g = min(GG, NBLK - b0)
        segs = []
        for s in range(NSEG):
            nch = int(run_len[gi, s]) // P
            segs.append((s, kk, nch))
            kk += nch
        gc = sum(x[2] for x in segs)
        groups.append((b0, g, segs))
    K = kk

    # per-core: index streams, per-instance dloc and instance schedule.
    # The instance schedule (which blocks each chunk touches) must be
    # IDENTICAL across cores (SPMD single program): merge spans over cores.
    # For each (group, seg, chunk) the set of possibly-touching blocks is
    # derived from per-core block spans; union over cores.
    inst_sets = [dict() for _ in range(ngrp)]   # (s, chunk) -> set(blocks)
    percore_edges = []
    for c in range(n_cores):
        ge = []
        for gi, (b0, g, segs) in enumerate(groups):
            for (s, cb, nch) in segs:
                if nch == 0:
                    continue
                cap = nch * P
                sp0 = np.zeros(cap, dtype=np.int64)
                sp1 = np.zeros(cap, dtype=np.int64)
                dp = np.full(cap, 200.0, dtype=np.float32)
                bl = np.full(cap, -1, dtype=np.int64)
                off = 0
                for bb in range(g):
                    sb0_, sb1_, db_ = percore[c][b0 + bb][s]
                    ns = len(sb0_)
                    sp0[off:off + ns] = sb0_
                    sp1[off:off + ns] = sb1_
                    dp[off:off + ns] = (db_ - (b0 + bb) * P)
                    bl[off:off + ns] = bb
                    off += ns
                for k in range(nch):
                    touched = set(bl[k * P:(k + 1) * P].tolist()) - {-1}
                    key = (s, cb + k)
                    inst_sets[gi].setdefault(key, set()).update(touched)
                ge.append((gi, s, cb, nch, (sp0, sp1), dp, bl))
        percore_edges.append(ge)

    # canonical instance order per group: seg-major, chunk-major, block asc
    inst_meta = []       # per group: list of (s, chunk_col, bb)
    for gi, (b0, g, segs) in enumerate(groups):
        il = []
        for (s, cb, nch) in segs:
            for k in range(nch):
                bbs = sorted(inst_sets[gi].get((s, cb + k), set()))
                if not bbs:
                    bbs = [g - 1]          # dummy all-pad chunk
                for bb in bbs:
                    il.append((s, cb + k, bb))
        inst_meta.append(il)
    I = sum(len(il) for il in inst_meta)

    # build per-group kernel schedules
    gsched = []
    icol = 0
    for gi, (b0, g, segs) in enumerate(groups):
        il = inst_meta[gi]
        # ed-matmul first/last per chunk; acc last per block
        by_chunk = {}
        by_blk = {}
        insts = []
        for j, (s, cl, bb) in enumerate(il):
            by_chunk.setdefault(cl, []).append(j)
            by_blk.setdefault(bb, []).append(j)
        for j, (s, cl, bb) in enumerate(il):
            insts.append(dict(
                cl=cl, bb=bb, icol=icol + j,
                ed_first=(j == by_chunk[cl][0]),
                ed_last=(j == by_chunk[cl][-1]),
                acc_last=(j == by_blk[bb][-1])))
        gc = sum(x[2] for x in segs)
        gsched.append(dict(b0=b0, g=g, segs=segs, gc=gc, insts=insts,
                           icol0=icol, ni=len(il)))
        icol += len(il)
    assert icol == I

    meta = dict(NSH=NSH, NBLK=NBLK, NSEG=NSEG, K=K, I=I, groups=gsched,
                spp=spp, seg_blks=seg_blks_)

    # per-core tensors
    out = []
    for c in range(n_cores):
        dlocI = np.full((P, I), 200.0, dtype=np.float32)
        gw_parts0, gw_parts1, goff = [], [], []
        go = 0
        chunk_dp = {}
        for (gi, s, cb, nch, sp, dp, bl) in percore_edges[c]:
            for k in range(nch):
                chunk_dp[(gi, s, cb + k)] = (dp[k * P:(k + 1) * P],
                                             bl[k * P:(k + 1) * P])
        for gi, (b0, g, segs) in enumerate(groups):
            slens = []
            for (gi2, s, cb, nch, sp, dp, bl) in percore_edges[c]:
                if gi2 != gi:
                    continue
                w0 = sp[0].reshape(-1, 16).T
                w1 = sp[1].reshape(-1, 16).T
                gw_parts0.append(np.tile(w0, (8, 1)).astype(np.int16))
                gw_parts1.append(np.tile(w1, (8, 1)).astype(np.int16))
                slens.append((s, go, len(sp[0])))
                go += len(sp[0]) // 16
            goff.append(slens)
            for inst in gsched[gi]["insts"]:
                cl, bb, ic = inst["cl"], inst["bb"], inst["icol"]
                # find seg of this chunk
                for (s, cb, nch) in segs:
                    if cb <= cl < cb + nch:
                        break
                dpk, blk_ = chunk_dp.get((gi, s, cl), (None, None))
                if dpk is None:
                    continue
                v = np.where(blk_ == bb, dpk, 200.0)
                dlocI[:, ic] = v
        ohT = (np.arange(P, dtype=np.float32)[:, None, None]
               == dlocI.T[None, :, :])
        out.append(dict(
            gidx0=np.concatenate(gw_parts0, axis=1),
            gidx1=np.concatenate(gw_parts1, axis=1),
            dloc=dlocI.astype(BF),
            ohT=np.ascontiguousarray(ohT.reshape(P, I * P)).astype(BF)))
    meta["goff"] = goff
    return meta, out


def _prep_inputs(inputs, n_cores=8):
    N, IN_DIM = inputs["state_wf"].shape
    HID = inputs["W0"].shape[1]
    VM = 16
    B = inputs["candidate_task_index"].shape[0] // VM
    meta, per_core_e = _schedule(inputs["edge_index"], N, n_cores)
    meta.update(N=N, IN_DIM=IN_DIM, HID=HID, VM=VM, B=B,
                NPAD=math.ceil(N / P) * P)

    f = lambda x: np.asarray(x, dtype=np.float32)
    W0, W1 = f(inputs["W0"]), f(inputs["W1"])
    w0big = np.concatenate(
        [W0, (W0 @ f(inputs["a_src0"]))[:, None],
         (W0 @ f(inputs["a_dst0"]))[:, None]], axis=1)
    w1big = np.concatenate(
        [W1, (W1 @ f(inputs["a_src1"]))[:, None],
         (W1 @ f(inputs["a_dst1"]))[:, None]], axis=1)
    swt = np.zeros((IN_DIM, meta["NPAD"]), dtype=np.float32)
    swt[:, :N] = f(inputs["state_wf"]).T
    NSH, NBLK = meta["NSH"], meta["NBLK"]
    spp, NSEG = meta["spp"], meta["NSEG"]
    common = dict(
        w0big=np.pad(np.concatenate(
            [np.pad(w0big.astype(np.float32), ((0, 14), (0, 0)))] * 3,
            axis=0), ((0, 32), (0, 0))),
        w1big=w1big.astype(np.float32),
        b0t=np.tile(f(inputs["b0"])[None, :], (P, 1)).astype(np.float32),
        b1t=np.tile(f(inputs["b1"])[None, :], (P, 1)).astype(np.float32),
        mw0=f(inputs["mW0"]),
        mw1=f(inputs["mW1"]).reshape(HID, 1),
        mb0=f(inputs["mb0"]).reshape(HID, 1),
        iota=np.tile(np.arange(P, dtype=np.float32)[None, :],
                     (P, 1)).astype(BF),
        iotac=np.arange(P, dtype=np.float32).reshape(P, 1).astype(BF),
        ident=np.eye(P, dtype=np.float32),
    )
    cand = inputs["candidate_task_index"].astype(np.int64)
    CPC = (B // n_cores) * VM
    CC = CPC // P
    meta["CC"] = CC
    in_maps = []
    for c in range(n_cores):
        m = dict(common)
        m.update(per_core_e[c])
        # full transposed state in this core's rotated walk order, packed
        # 7 projection stages deep across 126 partitions so the state loads
        # use the full DMA width (18-partition loads run ~4x slower)
        stages = []
        for sgi in range(NSEG):
            nst = (meta["seg_blks"][sgi] + 7) // 8
            for cpos in range(n_cores):
                csrc = (c + cpos) % n_cores
                for t in range(nst):
                    jn = min(8, meta["seg_blks"][sgi] - t * 8)
                    lo = csrc * NSH + sgi * SEGB * P + t * 8 * P
                    hi = min(lo + jn * P, csrc * NSH + NSH)
                    stages.append((lo, hi))
        nslab = (len(stages) + 2) // 3
        fat = np.zeros((128, nslab * 8 * P), dtype=np.float32)
        for st, (lo, hi) in enumerate(stages):
            k, sl = st % 3, st // 3
            fat[32 * k:32 * k + 18, sl * 8 * P:sl * 8 * P + (hi - lo)] = \
                swt[:, lo:hi]
        m["swtsh"] = fat
        m["cidx"] = cand[c * CPC:(c + 1) * CPC].reshape(P, CC).astype(np.int32)
        in_maps.append(m)
    return meta, in_maps


# ------------------------------------------------------------------ builder
def build(meta, n_cores=8):
    NSH, NBLK, NSEG = meta["NSH"], meta["NBLK"], meta["NSEG"]
    K, I, groups = meta["K"], meta["I"], meta["groups"]
    N, IN_DIM, HID = meta["N"], meta["IN_DIM"], meta["HID"]
    NPAD, CC, VM = meta["NPAD"], meta["CC"], meta["VM"]
    goff = meta["goff"]
    spp, seg_blks = meta["spp"], meta["seg_blks"]
    IWG = sum(ln // 16 for slens in goff for (_, _, ln) in slens)
    max_gc = max(gs["gc"] for gs in groups)
    max_ni = max(gs["ni"] for gs in groups)
    TOT = n_cores * sum(spp)
    NSTAGES = n_cores * sum((sb + 7) // 8 for sb in seg_blks)
    NSLAB = (NSTAGES + 2) // 3
    # L0 group after which z1 segment s is fully produced
    ag_after = [(s * SEGB + seg_blks[s] - 1) // GG for s in range(NSEG)]
    ag_after[-1] = len(groups) - 1

    SP = bool(int(os.environ.get("KERNEL_SP", "0")))
    nc = bacc.Bacc("TRN2", target_bir_lowering=False, debug=False,
                   enable_asserts=False, num_devices=n_cores)

    inp = {}
    for name, shape, dt in [
        ("swtsh", [P, NSLAB * 8 * P], F32),
        ("w0big", [P, HID + 2], F32),
        ("w1big", [HID, HID + 2], F32),
        ("b0t", [P, HID], F32), ("b1t", [P, HID], F32),
        ("mw0", [HID, HID], F32), ("mw1", [HID, 1], F32),
        ("mb0", [HID, 1], F32), ("iota", [P, P], BF16),
        ("iotac", [P, 1], BF16), ("ident", [P, P], F32),
        ("dloc", [P, I], BF16), ("ohT", [P, I * P], BF16),
        ("gidx0", [P, IWG], I16), ("gidx1", [P, IWG], I16),
        ("cidx", [P, CC], I32),
    ]:
        inp[name] = nc.dram_tensor(name, shape, dt, kind="ExternalInput")
    out_t = nc.dram_tensor("out", [P, CC], F32, kind="ExternalOutput")

    z1_sh = [nc.dram_tensor(f"z1sh{s}", [spp[s], TW], BF16,
                            kind="Internal") for s in range(NSEG)]
    z0_tb = [nc.dram_tensor(f"z0tb{s}", [n_cores * spp[s], TW], BF16,
                            kind="Internal") for s in range(NSEG)]
    z1_tb = [nc.dram_tensor(f"z1tb{s}", [n_cores * spp[s], TW], BF16,
                            kind="Internal", addr_space="Shared")
             for s in range(NSEG)]
    ed_tab = [nc.dram_tensor(f"ed{l}tab", [NBLK * P, 1], BF16, kind="Internal")
              for l in range(2)]
    sc_shard = nc.dram_tensor("scshard", [NBLK, P, 1], F32, kind="Internal")
    sc_full = nc.dram_tensor("scfull", [N, 1], F32, kind="Internal",
                             addr_space="Shared")

    with tile.TileContext(nc) as tc:
        with (
            tc.tile_pool(name="const", bufs=1) as cpool,
            tc.tile_pool(name="stream", bufs=2) as spool,
            tc.tile_pool(name="idxs", bufs=5) as ipool,
            tc.tile_pool(name="zrows", bufs=2) as zpool,
            tc.tile_pool(name="ohts", bufs=2) as opool,
            tc.tile_pool(name="zown", bufs=6) as znpool,
            tc.tile_pool(name="work", bufs=3) as wpool,
            tc.tile_pool(name="ohwp", bufs=6) as ohwpool,
            tc.tile_pool(name="stage", bufs=3) as stpool,
            tc.tile_pool(name="psacc", bufs=GG, space="PSUM") as psacc,
            tc.tile_pool(name="psaps", bufs=1, space="PSUM") as psaps,
            tc.tile_pool(name="pstp", bufs=1, space="PSUM") as pstp,
            tc.tile_pool(name="psproj", bufs=1, space="PSUM") as psproj,
            tc.tile_pool(name="pssc", bufs=1, space="PSUM") as pssc,
        ):
            sb = {}
            for name in ("w0big", "w1big", "b0t", "b1t", "mw0", "mw1", "mb0",
                         "iota", "iotac", "ident", "dloc", "cidx"):
                t = inp[name]
                dt = {"cidx": I32, "iota": BF16, "iotac": BF16,
                      "dloc": BF16}.get(name, F32)
                sb[name] = cpool.tile(list(t.shape), dt, tag=name, name=name)
                nc.sync.dma_start(sb[name][:], t[:])

            # ===== phase 0: local FULL z0 projection (no collective) ====
            # Each core projects the entire node table into its own rotated
            # (seg, cpos) layout; own blocks sit first in each segment, so
            # self-loop and ed0 addresses are core-independent.
            def seg_allgather(s):
                nc.gpsimd.collective_compute(
                    "AllGather", ALU.bypass,
                    replica_groups=[list(range(n_cores))],
                    ins=[z1_sh[s][:].flatten().opt()],
                    outs=[z1_tb[s][:].flatten().opt()])

            NSTG = 8
            stages = []        # (seg, cpos, stage-in-(seg,cpos), jn)
            for sgi in range(NSEG):
                nst = (seg_blks[sgi] + 7) // 8
                for cpos in range(n_cores):
                    for t in range(nst):
                        stages.append((sgi, cpos, t,
                                       min(8, seg_blks[sgi] - t * 8)))
            nslab = (len(stages) + 2) // 3
            for sl in range(nslab):
                swsh_sb = spool.tile([P, NSTG * P], F32, tag="swt",
                                     name="swsh")
                nc.sync.dma_start(
                    swsh_sb[:],
                    inp["swtsh"][:, sl * NSTG * P:(sl + 1) * NSTG * P])
                for k in range(3):
                    st = sl * 3 + k
                    if st >= len(stages):
                        break
                    sgi, cpos, t, jn = stages[st]
                    stg = stpool.tile([P, NSTG * TW], BF16, tag="z0st",
                                      name="z0stg")
                    nc.vector.memset(stg[:], 1.0)
                    estg = stpool.tile([P, NSTG], BF16, tag="edst",
                                       name="edstg")
                    for j in range(jn):
                        ps = psacc.tile([P, HID + 2], F32, tag="acc",
                                        name="z0ps")
                        nc.tensor.matmul(
                            ps[:],
                            swsh_sb[32 * k:32 * k + 18,
                                    j * P:(j + 1) * P],
                            sb["w0big"][32 * k:32 * k + 18, :],
                            start=True, stop=True)
                        if j % 2 == 0:
                            nc.scalar.copy(
                                stg[:, j * TW:j * TW + HID + 1],
                                ps[:, :HID + 1])
                        else:
                            nc.vector.tensor_copy(
                                stg[:, j * TW:j * TW + HID + 1],
                                ps[:, :HID + 1])
                        if cpos == 0:
                            nc.vector.tensor_copy(
                                estg[:, j:j + 1],
                                ps[:, HID + 1:HID + 2])
                    r0 = cpos * spp[sgi] + t * NSTG * P
                    nc.sync.dma_start(
                        z0_tb[sgi][r0:r0 + jn * P].rearrange(
                            "(p j) c -> p (j c)", j=jn),
                        stg[:, :jn * TW])
                    if cpos == 0:
                        r0g = (sgi * SEGB + t * NSTG) * P
                        nc.sync.dma_start(
                            ed_tab[0][r0g:r0g + jn * P].rearrange(
                                "(j p) c -> p j c", p=P),
                            estg[:, :jn].rearrange("p (j c) -> p j c",
                                                   c=1))

            # ======================= GAT layers ============================
            for layer in range(2):
                btile = sb["b0t"] if layer == 0 else sb["b1t"]
                z1_ag_next = 0
                for gi_, gs in enumerate(groups):
                    gb0, g, gc, ni = gs["b0"], gs["g"], gs["gc"], gs["ni"]
                    ic0 = gs["icol0"]
                    zr = zpool.tile([P, max_gc, TW], BF16, tag="zr", name="zr")
                    oht = opool.tile([P, max_ni * P], BF16, tag="oht",
                                     name="oht")
                    nc.sync.dma_start(oht[:, :ni * P],
                                      inp["ohT"][:, ic0 * P:(ic0 + ni) * P])
                    edc = ipool.tile([P, GG], BF16, tag="edc", name="edc")
                    nc.sync.dma_start(
                        edc[:, :g],
                        ed_tab[layer][gb0 * P:(gb0 + g) * P].rearrange(
                            "(j p) c -> p (j c)", p=P))
                    # --- gathers (per index segment) ---
                    for (s, goff_s, ln) in goff[gi_]:
                        gidx = ipool.tile([P, max(ln // 16, 1)], I16,
                                          tag="gidx", name="gidxt")
                        gsrc = inp["gidx0"] if layer == 0 else inp["gidx1"]
                        nc.sync.dma_start(
                            gidx[:, :ln // 16],
                            gsrc[:, goff_s:goff_s + ln // 16])
                        for (s_, cb, nch) in gs["segs"]:
                            if s_ == s:
                                break
                        assert s_ == s and nch == ln // P
                        ztab = z0_tb[s] if layer == 0 else z1_tb[s]
                        nc.gpsimd.dma_gather(
                            out_ap=zr[:, cb - gs["segs"][0][1]:
                                      cb - gs["segs"][0][1] + nch, :],
                            in_ap=ztab[:],
                            idxs_ap=gidx[:, :ln // 16],
                            num_idxs=ln, num_idxs_reg=ln, elem_size=TW,
                            single_packet=SP)
                    # --- self-loop diagonal per block (opens psum accum) ---
                    bps = {}
                    for bi in range(g):
                        b = gb0 + bi
                        bsg = b // SEGB
                        br = (b - bsg * SEGB) * P
                        zo = znpool.tile([P, TW], BF16, tag="zo", name="zo")
                        if layer == 0:
                            jl = b - bsg * SEGB
                            jn0 = 8 if bsg < NSEG - 1 else seg_blks[-1]
                            t0 = jl // 8
                            jj = jl % 8
                            base = t0 * 8 * P
                            nc.sync.dma_start(
                                zo[:],
                                z0_tb[bsg][base:base + jn0 * P].rearrange(
                                    "(p j) c -> p j c", j=jn0)[:, jj, :])
                        else:
                            nc.sync.dma_start(zo[:], z1_sh[bsg][br:br + P])
                        sxa = wpool.tile([P, 1], F32, tag="sxa", name="sxa")
                        nc.vector.tensor_tensor(
                            out=sxa[:], in0=zo[:, HID:HID + 1],
                            in1=edc[:, bi:bi + 1], op=ALU.add)
                        sxl = wpool.tile([P, 1], F32, tag="sxl", name="sxl")
                        nc.vector.scalar_tensor_tensor(
                            out=sxl[:], in0=sxa[:], scalar=NEG_SLOPE,
                            in1=sxa[:], op0=ALU.mult, op1=ALU.max)
                        sx = wpool.tile([P, 1], BF16, tag="sx", name="sx")
                        nc.scalar.activation(out=sx[:], in_=sxl[:],
                                             func=ACTF.Exp)
                        dg = ohwpool.tile([P, P], BF16, tag="ohw", name="dg")
                        nc.vector.scalar_tensor_tensor(
                            out=dg[:], in0=sb["iota"][:],
                            scalar=sb["iotac"][:],
                            in1=sx[:].to_broadcast([P, P]),
                            op0=ALU.is_equal, op1=ALU.mult)
                        bps[bi] = psacc.tile([P, HID + 2], F32, tag="acc",
                                             name="bps")
                        nc.tensor.matmul(
                            bps[bi][:], dg[:], zo[:, 0:HID + 2],
                            start=True, stop=(len([i for i in gs["insts"]
                                                   if i["bb"] == bi]) == 0),
                            skip_group_check=True)
                    # --- pass 1: ed per edge via one-hot matmuls ---
                    aps = psaps.tile([P, max_gc], F32, tag="aps", name="aps")
                    for inst in gs["insts"]:
                        cl = inst["cl"] - gs["segs"][0][1]
                        lc = inst["icol"] - ic0
                        nc.tensor.matmul(
                            aps[:, cl:cl + 1],
                            oht[:, lc * P:(lc + 1) * P],
                            edc[:, inst["bb"]:inst["bb"] + 1],
                            start=inst["ed_first"], stop=inst["ed_last"],
                            skip_group_check=True)
                    # --- alpha = es + ed, lrelu, exp (batched per group) ---
                    tse = wpool.tile([P, max_gc], F32, tag="tse", name="tse")
                    nc.vector.tensor_tensor(
                        out=tse[:, :gc], in0=aps[:, :gc],
                        in1=zr[:, :gc, HID:HID + 1].rearrange(
                            "p a b -> p (a b)"),
                        op=ALU.add)
                    lr = wpool.tile([P, max_gc], F32, tag="lr", name="lr")
                    nc.vector.scalar_tensor_tensor(
                        out=lr[:, :gc], in0=tse[:, :gc], scalar=NEG_SLOPE,
                        in1=tse[:, :gc], op0=ALU.mult, op1=ALU.max)
                    exc = wpool.tile([P, max_gc], BF16, tag="exc", name="exc")
                    nc.scalar.activation(out=exc[:, :gc], in_=lr[:, :gc],
                                         func=ACTF.Exp)
                    # --- pass 2: weighted one-hot + scatter matmuls ---
                    if layer == 0:
                        z1stg = stpool.tile([P, GG * TW], BF16, tag="z1st",
                                            name="z1stg")
                        nc.vector.memset(z1stg[:], 1.0)
                        ed1stg = stpool.tile([P, GG], BF16, tag="ed1st",
                                             name="ed1stg")
                    else:
                        scstg = stpool.tile([1, GG * P], F32, tag="scst",
                                            name="scstg")
                    for inst in gs["insts"]:
                        cl = inst["cl"] - gs["segs"][0][1]
                        ohw = ohwpool.tile([P, P], BF16, tag="ohw",
                                           name="ohw")
                        nc.vector.scalar_tensor_tensor(
                            out=ohw[:], in0=sb["iota"][:],
                            scalar=sb["dloc"][:, inst["icol"]:
                                              inst["icol"] + 1],
                            in1=exc[:, cl:cl + 1].to_broadcast([P, P]),
                            op0=ALU.is_equal, op1=ALU.mult)
                        nc.tensor.matmul(
                            bps[inst["bb"]][:], ohw[:],
                            zr[:, cl:cl + 1, 0:HID + 2].squeeze(),
                            start=False, stop=inst["acc_last"],
                            skip_group_check=True)
                    # --- epilogues (all chunks of the group are done) ---
                    for bi in range(g):
                        b = gb0 + bi
                        pb = bps[bi]
                        rc = wpool.tile([P, 1], F32, tag="rc", name="rc")
                        nc.vector.reciprocal(rc[:], pb[:, HID + 1:HID + 2])
                        y = wpool.tile([P, HID], F32, tag="y", name="y")
                        nc.vector.scalar_tensor_tensor(
                            out=y[:], in0=pb[:, :HID], scalar=rc[:],
                            in1=btile[:], op0=ALU.mult, op1=ALU.add)
                        e_t = wpool.tile([P, HID], F32, tag="e_t", name="e_t")
                        r_t = wpool.tile([P, HID], F32, tag="r_t", name="r_t")
                        nc.scalar.activation(out=e_t[:], in_=y[:],
                                             func=ACTF.Exp)
                        nc.scalar.activation(out=r_t[:], in_=y[:],
                                             func=ACTF.Relu)
                        hp1 = wpool.tile([P, HID], F32, tag="hp1", name="hp1")
                        nc.vector.scalar_tensor_tensor(
                            out=hp1[:], in0=e_t[:], scalar=1.0, in1=r_t[:],
                            op0=ALU.min, op1=ALU.add)      # elu(y) + 1
                        tp = pstp.tile([P, P], F32, tag="tp", name="tp")
                        nc.tensor.transpose(tp[:], hp1[:], sb["ident"][:])
                        hT = wpool.tile([P, HID], F32, tag="hT", name="hT")
                        nc.scalar.activation(out=hT[:], in_=tp[:],
                                             func=ACTF.Copy, bias=-1.0)
                        if layer == 0:
                            zps = psproj.tile([P, HID + 2], F32, tag="proj",
                                              name="zps")
                            nc.tensor.matmul(zps[:], hT[:], sb["w1big"][:],
                                             start=True, stop=True,
                                             skip_group_check=True)
                            nc.scalar.copy(
                                z1stg[:, bi * TW:bi * TW + HID + 1],
                                zps[:, :HID + 1])
                            nc.vector.tensor_copy(
                                ed1stg[:, bi:bi + 1],
                                zps[:, HID + 1:HID + 2])
                        else:
                            mps = psproj.tile([P, HID], F32, tag="proj",
                                              name="mps")
                            nc.tensor.matmul(mps[:], sb["mw0"][:], hT[:],
                                             start=True, stop=True,
                                             skip_group_check=True)
                            m1 = wpool.tile([P, HID], F32, tag="m1", name="m1")
                            nc.scalar.activation(out=m1[:], in_=mps[:],
                                                 func=ACTF.Relu,
                                                 bias=sb["mb0"][:])
                            sps = pssc.tile([1, P], F32, tag="sc",
                                            name="sps")
                            nc.tensor.matmul(sps[:], sb["mw1"][:], m1[:],
                                             start=True, stop=True,
                                             skip_group_check=True)
                            nc.scalar.copy(scstg[:, bi * P:(bi + 1) * P],
                                           sps[:])
                    if layer == 0:
                        gsg = gb0 // SEGB
                        gr0 = (gb0 - gsg * SEGB) * P
                        nc.sync.dma_start(
                            z1_sh[gsg][gr0:gr0 + g * P].rearrange(
                                "(j p) c -> p j c", p=P),
                            z1stg[:, :g * TW].rearrange("p (j c) -> p j c",
                                                        c=TW))
                        nc.sync.dma_start(
                            ed_tab[1][gb0 * P:(gb0 + g) * P].rearrange(
                                "(j p) c -> p j c", p=P),
                            ed1stg[:, :g].rearrange("p (j c) -> p j c", c=1))
                        while (z1_ag_next < NSEG
                               and gi_ >= ag_after[z1_ag_next]):
                            seg_allgather(z1_ag_next)
                            z1_ag_next += 1
                    else:
                        nc.sync.dma_start(sc_shard[gb0:gb0 + g],
                                          scstg[:, :g * P])

            # ================= scores + candidate softmax ==================
            nc.gpsimd.collective_compute(
                "AllGather", ALU.bypass,
                replica_groups=[list(range(n_cores))],
                ins=[sc_shard[:].flatten()[0:NSH].opt()],
                outs=[sc_full[:].flatten().opt()])
            scg = wpool.tile([P, CC], F32, tag="scg", name="scg")
            for c in range(CC):
                nc.gpsimd.indirect_dma_start(
                    out=scg[:, c:c + 1], out_offset=None, in_=sc_full[:],
                    in_offset=IndirectOffsetOnAxis(
                        ap=sb["cidx"][:, c:c + 1], axis=0))
            NG = CC // VM
            pex = wpool.tile([P, CC], F32, tag="pex", name="pex")
            nc.scalar.activation(out=pex[:], in_=scg[:], func=ACTF.Exp)
            ssum = wpool.tile([P, NG], F32, tag="ssum", name="ssum")
            nc.vector.tensor_reduce(
                out=ssum[:], in_=pex[:].rearrange("p (g v) -> p g v", v=VM),
                axis=mybir.AxisListType.X, op=ALU.add)
            rcg = wpool.tile([P, NG], F32, tag="rcg", name="rcg")
            nc.vector.reciprocal(rcg[:], ssum[:])
            pi = wpool.tile([P, CC], F32, tag="pi", name="pi")
            for g_ in range(NG):
                nc.vector.tensor_scalar(
                    out=pi[:, g_ * VM:(g_ + 1) * VM],
                    in0=pex[:, g_ * VM:(g_ + 1) * VM],
                    scalar1=rcg[:, g_:g_ + 1], scalar2=0.0,
                    op0=ALU.mult, op1=ALU.add)
            nc.sync.dma_start(out_t[:], pi[:])

    return nc


# ------------------------------------------------------------------- kernel
def kernel(**inputs):
    n_cores = 8
    meta, in_maps = _prep_inputs(inputs, n_cores)
    nc = build(meta, n_cores)
    nc.compile()
    res = run_bass_kernel_spmd(
        nc, in_maps, core_ids=list(range(n_cores)),
        trace=bool(int(os.environ.get("KERNEL_TRACE", "0"))))
    kernel.last_results = res
    kernel.last_meta = meta
    VM = meta["VM"]
    outs = [res.results[c]["out"].reshape(-1, VM) for c in range(n_cores)]
    return np.concatenate(outs, axis=0).astype(np.float32)

